# revision 46
# baseline (speedup 1.0000x reference)
"""AttnBlock (GroupNorm + 1x1-conv QKV + single-head spatial attention + proj
+ residual) on 8 Trainium2 NeuronCores.

Sharding: pure data-parallel over batch - 16 samples / 8 cores = 2 samples per
core; weights broadcast. No collectives; gather on host.

Algebraic restructuring (exact, cuts PE work ~22% vs the naive formulation):
  scores  = (Wq h)^T (Wk h) = h^T M h   with M  = Wq^T Wk   (host-precomputed)
  y_attn  = P (Wv h) p~     = W2 h p~   with W2 = P Wv      (host-precomputed)
so the kernel runs only 4 GEMM families per sample:
  Gp = M^T h          (C x C x N)      [scores moving operand]
  sT = h^T Gp         (N x N x C)      -> e = exp(s * C^-0.5)  (bf16)
  uT = h^T W2^T       (C x C x N)      [attention value rows]
  o  = uT^T e         (C x N x N)      -> y = x + o * (1/S)
The proj GEMM disappears entirely. qkv_b / proj_b are assumed zero (true for
this problem's inputs; falls back to a NumPy path otherwise); norm_w / norm_b
handled generally.

Internal storage is bf16 (PE streams bf16 at the same 1 col/cycle as fp32r,
but bf16 halves SBUF/DMA). Softmax denominators S via ones-column matmul; 1/S
broadcast to 128 partitions via a rank-1 PE matmul (no DRAM bounce). Warmup
matmuls on a memset tile run during the input-DMA window so the HAM throttle
reaches full clock before real work starts. Input DMAs are kept to 13 large
descriptors (weights pre-tiled host-side, consts merged) so the DMA-semaphore
pool never recycles during the critical startup window; GroupNorm vector ops
are batched across all 4 channel tiles to shorten the stats critical path.
"""

import numpy as np
import ml_dtypes

import concourse.bass as bass
import concourse.tile as tile
from concourse import bacc, mybir
from concourse.bass_utils import run_bass_kernel_spmd

B, C, H, W = 16, 512, 32, 32
N = H * W              # 1024 spatial positions
G = 32                 # groups
GS = C // G            # 16 channels per group
NCORES = 8
SPC = B // NCORES      # samples per core
EPS = 1e-6
SCALE = float(C) ** -0.5
KT = C // 128          # 4 channel tiles of 128
NT = N // 128          # 8 spatial tiles of 128
NH = N // 512          # 2 free-dim halves of 512
NWARM = 38             # PE warmup matmuls (HAM ramp during DMA window)

F32 = mybir.dt.float32
F32R = mybir.dt.float32r
BF16 = mybir.dt.bfloat16

_BUILD_CACHE = {}
LAST_RESULT = None  # BassKernelResults of the most recent run (for test harness)


def _build():
    nc = bacc.Bacc("TRN2", target_bir_lowering=False, debug=False)

    # x pre-tiled host-side to [SPC, 128, KT, N]; f32 for the residual add,
    # bf16 copy for GroupNorm stats+apply (halves the startup-critical DMA)
    xt_ext = nc.declare_dram_parameter("xt", [SPC, 128, KT, N], F32, isOutput=False)
    xbf_ext = nc.declare_dram_parameter("xbf", [SPC, 128, KT, N], BF16, isOutput=False)
    # weights pre-tiled host-side: [128, KT*C] with (p, kt*C + c) = W[kt*128+p, c]
    scw_ext = nc.declare_dram_parameter("scw", [128, KT * C], BF16, isOutput=False)
    vw2T_ext = nc.declare_dram_parameter("vw2T", [128, KT * C], BF16, isOutput=False)
    # merged consts: cols 0-3 norm_w, 4-7 norm_b, 8-15 group indicator, 16 ones
    cst_ext = nc.declare_dram_parameter("cst17", [128, 17], F32, isOutput=False)
    indT_ext = nc.declare_dram_parameter("ind16T", [8, 128], F32, isOutput=False)
    onesf_ext = nc.declare_dram_parameter("ones_f", [128], F32R, isOutput=False)
    y_ext = nc.declare_dram_parameter("y", [SPC, C, N], F32, isOutput=True)

    Identity = mybir.ActivationFunctionType.Identity
    Copy = mybir.ActivationFunctionType.Copy
    Exp = mybir.ActivationFunctionType.Exp
    Sqrt = mybir.ActivationFunctionType.Sqrt
    mult = mybir.AluOpType.mult
    add = mybir.AluOpType.add

    with tile.TileContext(nc) as tc:
        with (
            tc.tile_pool(name="wpool", bufs=1) as wpool,
            tc.tile_pool(name="cpool", bufs=1) as cpool,
            tc.tile_pool(name="xpool", bufs=2) as xpool,
            tc.tile_pool(name="hpool", bufs=2) as hpool,
            tc.tile_pool(name="gpool", bufs=1) as gpool,
            tc.tile_pool(name="upool", bufs=1) as upool,
            tc.tile_pool(name="epool", bufs=1) as epool,
            tc.tile_pool(name="opool", bufs=4) as opool,
            tc.tile_pool(name="gnpool", bufs=2) as gnpool,
            tc.tile_pool(name="spool", bufs=2) as spool,
            tc.tile_pool(name="ps", bufs=8, space="PSUM") as ps,
        ):
            # ---- input DMAs: few large descriptors, split across two queues.
            # Startup-critical: xbf0 (1MB) + weights only; the f32 x copies and
            # sample-1 tensors are gated on h0 (markers below) so xbf0 gets the
            # full DMA bandwidth. ----
            x_tiles = [
                xpool.tile([128, KT, N], F32, tag="x", name=f"x_sb{s}")
                for s in range(SPC)
            ]
            xbf_tiles = [
                xpool.tile([128, KT, N], BF16, tag="xbf", name=f"xbf_sb{s}")
                for s in range(SPC)
            ]
            nc.sync.dma_start(
                out=xbf_tiles[0][:, 0:2, :], in_=xbf_ext.ap()[0][:, 0:2, :]
            )
            nc.gpsimd.dma_start(
                out=xbf_tiles[0][:, 2:KT, :], in_=xbf_ext.ap()[0][:, 2:KT, :]
            )

            cst_sb = cpool.tile([128, 17], F32)
            nc.sync.dma_start(out=cst_sb, in_=cst_ext.ap())
            nw_sb = cst_sb[:, 0:4]
            nb_sb = cst_sb[:, 4:8]
            indT_sb = cpool.tile([8, 128], F32)
            nc.sync.dma_start(out=indT_sb, in_=indT_ext.ap())
            onesf_row = cpool.tile([1, 128], F32R)
            nc.sync.dma_start(out=onesf_row, in_=onesf_ext.ap().unsqueeze(0))

            scw_sb = wpool.tile([128, KT, C], BF16)
            nc.gpsimd.dma_start(
                out=scw_sb, in_=scw_ext.ap()
            )
            vw2T_sb = wpool.tile([128, KT, C], BF16)
            # gated (marker-dependent) loads, emitted later in the schedule

            # small derived consts (DVE casts off the merged tile)
            ind_sb = cst_sb[:, 8:16]
            onesb_col = cpool.tile([128, 1], BF16)
            nc.vector.tensor_copy(onesb_col, cst_sb[:, 16:17])

            # ---- ACT table warmups ----
            eps_sb = cpool.tile([8, 1], F32)
            nc.vector.memset(eps_sb, EPS)
            warm_sb = cpool.tile([8, 1], F32)
            nc.scalar.activation(out=warm_sb, in_=eps_sb, func=Identity)
            nc.scalar.activation(out=warm_sb, in_=eps_sb, func=Sqrt)
            nc.scalar.activation(out=warm_sb, in_=eps_sb, func=Exp)

            # ---- PE warmup: no-dependency matmuls on a memset tile so the
            # HAM throttle ramps to full clock while x0 is still loading.
            # Emitted in chunks interleaved with the GroupNorm matmuls so the
            # PE stays busy through the gn small-op chain. ----
            warm_f = cpool.tile([128, 512], F32)
            nc.vector.memset(warm_f, 0.25)
            warm_mm = cpool.tile([128, 512], BF16)
            nc.vector.tensor_copy(warm_mm, warm_f)

            def warm(n):
                for _ in range(n):
                    pw = ps.tile([128, 512], F32, tag="mm")
                    nc.tensor.matmul(
                        pw, warm_mm[:, 0:128], warm_mm, start=True, stop=True
                    )

            def gn_stats_pre(s):
                """Per-partition moment DVE work for sample s -> s2f tile.

                All cross-kt vector work is batched into single strided ops to
                keep the serial small-op chain short."""
                x_sb = xbf_tiles[s]
                s2f = gnpool.tile([128, KT, 2], F32, tag="s2f", name=f"s2f{s}")
                for kt in range(KT):
                    stats = gnpool.tile(
                        [128, 2, 6], F32, tag=f"stats{kt}", name=f"stats{s}_{kt}"
                    )
                    for sg in range(2):
                        nc.vector.bn_stats(
                            out=stats[:, sg, :],
                            in_=x_sb[:, kt, sg * 512 : (sg + 1) * 512],
                        )
                    nc.vector.bn_aggr(out=s2f[:, kt, :], in_=stats)
                # turn [mean, var] into [mean, E[x^2]] in place, batched
                msq2 = gnpool.tile([128, KT], F32, tag="msq2", name=f"msq2{s}")
                nc.vector.tensor_mul(msq2, s2f[:, :, 0], s2f[:, :, 0])
                nc.vector.tensor_add(s2f[:, :, 1], s2f[:, :, 1], msq2)
                return s2f

            def gn_stats(s, s2f=None):
                """GroupNorm stats -> mr (8,KT,2) F32 [group mean, rstd]."""
                if s2f is None:
                    s2f = gn_stats_pre(s)
                # ind_sb carries 1/16 so ps_gs is already the group mean
                ps_gs = ps.tile([8, KT, 2], F32, tag="mm", name=f"ps_gs{s}")
                nc.tensor.matmul(ps_gs, ind_sb, s2f, start=True, stop=True)
                mr = gnpool.tile([8, KT, 2], F32, tag="mr", name=f"mr{s}")
                msq = gnpool.tile([8, KT], F32, tag="msq", name=f"msq{s}")
                nc.vector.tensor_copy(mr[:, :, 0], ps_gs[:, :, 0])
                nc.vector.tensor_mul(msq, mr[:, :, 0], mr[:, :, 0])
                nc.vector.tensor_sub(mr[:, :, 1], ps_gs[:, :, 1], msq)
                nc.scalar.activation(
                    out=mr[:, :, 1], in_=mr[:, :, 1], func=Sqrt, bias=eps_sb
                )
                nc.vector.reciprocal(mr[:, :, 1], mr[:, :, 1])
                return mr

            def gn_apply(s, mr, engines):
                """Broadcast stats to channels and apply x*scale+bias -> h bf16.

                engines: per-kt 'v' (DVE), 'a' (ACT) or 'g' (GpSimd)."""
                x_sb = xbf_tiles[s]
                h_sb = hpool.tile([128, KT, N], BF16, tag="h", name=f"h{s}")
                ps_bc = ps.tile([128, KT, 2], F32, tag="mm", name=f"ps_bc{s}")
                nc.tensor.matmul(ps_bc, indT_sb, mr, start=True, stop=True)
                scb = gnpool.tile([128, KT, 2], F32, tag="scb", name=f"scb{s}")
                nc.vector.tensor_mul(scb[:, :, 0], ps_bc[:, :, 1], nw_sb)
                nc.vector.tensor_mul(scb[:, :, 1], ps_bc[:, :, 0], scb[:, :, 0])
                nc.vector.tensor_sub(scb[:, :, 1], nb_sb, scb[:, :, 1])
                for kt in range(KT):
                    args = dict(
                        out=h_sb[:, kt, :],
                        in0=x_sb[:, kt, :],
                        scalar1=scb[:, kt, 0:1],
                        scalar2=scb[:, kt, 1:2],
                        op0=mult,
                        op1=add,
                    )
                    if engines[kt] == "v":
                        nc.vector.tensor_scalar(**args)
                    elif engines[kt] == "g":
                        nc.gpsimd.tensor_scalar(**args)
                    else:
                        nc.scalar.activation(
                            out=h_sb[:, kt, :], in_=x_sb[:, kt, :],
                            func=Identity, scale=scb[:, kt, 0:1],
                            bias=scb[:, kt, 1:2],
                        )
                return h_sb

            def gp_gemm(s, h_sb):
                """Gp = M^T h -> [128(c'), KT, N] bf16. Moves split DVE/ACT."""
                gp_sb = gpool.tile([128, KT, N], BF16, tag="gp", name=f"gp{s}")
                for obl in range(KT):
                    for ih in range(NH):
                        pm = ps.tile([128, 512], F32, tag="mm")
                        for kt in range(KT):
                            nc.tensor.matmul(
                                pm,
                                scw_sb[:, kt, obl * 128 : (obl + 1) * 128],
                                h_sb[:, kt, ih * 512 : (ih + 1) * 512],
                                start=(kt == 0),
                                stop=(kt == KT - 1),
                            )
                        if obl % 2 == 0:
                            nc.vector.tensor_copy(
                                gp_sb[:, obl, ih * 512 : (ih + 1) * 512], pm
                            )
                        else:
                            nc.scalar.activation(
                                out=gp_sb[:, obl, ih * 512 : (ih + 1) * 512],
                                in_=pm, func=Identity,
                            )
                return gp_sb

            def scores_exp(s, h_sb, gp_sb):
                """e = exp(scale * h^T Gp) -> [128(j), NT, N(i)] bf16 (ACT)."""
                e_sb = epool.tile([128, NT, N], BF16, tag="e", name=f"e{s}")
                for jt in range(NT):
                    for ih in range(NH):
                        pm = ps.tile([128, 512], F32, tag="mm")
                        for kt in range(KT):
                            nc.tensor.matmul(
                                pm,
                                h_sb[:, kt, jt * 128 : (jt + 1) * 128],
                                gp_sb[:, kt, ih * 512 : (ih + 1) * 512],
                                start=(kt == 0),
                                stop=(kt == KT - 1),
                            )
                        nc.scalar.activation(
                            out=e_sb[:, jt, ih * 512 : (ih + 1) * 512],
                            in_=pm,
                            func=Exp,
                            scale=SCALE,
                        )
                return e_sb

            def ut_gemm(s, h_sb):
                """uT = h^T W2^T -> [128(j), NT, C] bf16. PSUM moves on ACT."""
                ut_sb = upool.tile([128, NT, C], BF16, tag="ut", name=f"ut{s}")
                for nt in range(NT):
                    pm = ps.tile([128, 512], F32, tag="mm")
                    for kt in range(KT):
                        nc.tensor.matmul(
                            pm,
                            h_sb[:, kt, nt * 128 : (nt + 1) * 128],
                            vw2T_sb[:, kt, :],
                            start=(kt == 0),
                            stop=(kt == KT - 1),
                        )
                    nc.scalar.activation(
                        out=ut_sb[:, nt, :], in_=pm, func=Identity
                    )
                return ut_sb

            def softmax_S(s, e_sb):
                """S = ones^T e -> 1/S on one partition (DVE recip)."""
                recipSf = spool.tile([1, N], F32, tag="recipSf", name=f"recipSf{s}")
                recipS = spool.tile([1, N], F32R, tag="recipS", name=f"recipS{s}")
                for ih in range(NH):
                    pS = ps.tile([1, 512], F32, tag="mm")
                    for jt in range(NT):
                        nc.tensor.matmul(
                            pS,
                            onesb_col,
                            e_sb[:, jt, ih * 512 : (ih + 1) * 512],
                            start=(jt == 0),
                            stop=(jt == NT - 1),
                        )
                    nc.vector.reciprocal_approx_fast(
                        out=recipSf[:, ih * 512 : (ih + 1) * 512], in_=pS
                    )
                nc.vector.tensor_copy(recipS, recipSf)
                return recipS

            def softmax_bc(s, recipS):
                """1/S broadcast to 128 partitions via PE rank-1 matmul
                -> rsb [128, N] f32 in SBUF (ACT copies)."""
                rsb = spool.tile([128, N], F32, tag="rsb", name=f"rsb{s}")
                for ih in range(NH):
                    pbc = ps.tile([128, 512], F32, tag="mm")
                    nc.tensor.matmul(
                        pbc,
                        onesf_row,
                        recipS[:, ih * 512 : (ih + 1) * 512],
                        start=True,
                        stop=True,
                    )
                    nc.scalar.activation(
                        out=rsb[:, ih * 512 : (ih + 1) * 512], in_=pbc,
                        func=Identity,
                    )
                return rsb

            def attn_out_resid(s, ut_sb, e_sb, rsb, x_sb):
                """y = x + (uT^T e) * rS, chunkwise: DVE mult+add, y DMAs
                alternate between the gpsimd and sync queues. The last chunk
                of the last sample is processed in half-width pieces so the
                post-matmul trail (mult+add+DMA) is shorter."""
                for ct in range(KT):
                    for ih in range(NH):
                        pm = ps.tile([128, 512], F32, tag="mm")
                        for jt in range(NT):
                            nc.tensor.matmul(
                                pm,
                                ut_sb[:, jt, ct * 128 : (ct + 1) * 128],
                                e_sb[:, jt, ih * 512 : (ih + 1) * 512],
                                start=(jt == 0),
                                stop=(jt == NT - 1),
                            )
                        oy = opool.tile(
                            [128, 512], F32, tag="oy", name=f"oy{s}_{ct}_{ih}"
                        )
                        last = s == SPC - 1 and ct == KT - 1 and ih == NH - 1
                        pieces = 2 if last else 1
                        pw = 512 // pieces
                        for pc in range(pieces):
                            pslc = slice(pc * pw, (pc + 1) * pw)
                            yslc = slice(
                                ih * 512 + pc * pw, ih * 512 + (pc + 1) * pw
                            )
                            nc.vector.tensor_mul(
                                oy[:, pslc], pm[:, pslc], rsb[:, yslc]
                            )
                            xc = x_sb[:, ct, yslc]
                            nc.vector.tensor_add(xc, oy[:, pslc], xc)
                            dma_eng = (
                                nc.gpsimd
                                if (ct * NH + ih + pc) % 2 == 0
                                else nc.sync
                            )
                            dma_eng.dma_start(
                                out=y_ext.ap()[
                                    s, ct * 128 : (ct + 1) * 128, yslc
                                ],
                                in_=xc,
                            )

            # ---- schedule: two samples, phases interleaved ----
            warm(30)
            mr0 = gn_stats(0)
            warm(6)
            h0 = gn_apply(0, mr0, engines="vagv")
            warm(6)
            # Remaining transfers (f32 x, sample-1 bf16 x, vw2T) gated on h0
            # via tiny marker writes, so xbf0 gets full HBM bandwidth during
            # the startup-critical window (sample-0 GroupNorm gates all PE
            # work). Marker values are overwritten by the DMAs. Order matters:
            # xbf1 first (bn1 needs it ~35us), then vw2T (~45us), then f32 x
            # (residual adds, ~60us / ~105us).
            for t in (x_tiles[0], x_tiles[1], xbf_tiles[1], vw2T_sb):
                nc.gpsimd.tensor_copy(t[0:1, 0, 0:1], h0[0:1, 0, 0:1])
            nc.gpsimd.tensor_copy(xbf_tiles[1][0:1, 2, 0:1], h0[0:1, 0, 0:1])
            nc.sync.dma_start(
                out=xbf_tiles[1][:, 0:2, :], in_=xbf_ext.ap()[1][:, 0:2, :]
            )
            nc.gpsimd.dma_start(
                out=xbf_tiles[1][:, 2:KT, :], in_=xbf_ext.ap()[1][:, 2:KT, :]
            )
            nc.gpsimd.dma_start(out=vw2T_sb, in_=vw2T_ext.ap())
            nc.sync.dma_start(out=x_tiles[0], in_=xt_ext.ap()[0])
            nc.gpsimd.dma_start(out=x_tiles[1], in_=xt_ext.ap()[1])
            gp0 = gp_gemm(0, h0)
            e0 = scores_exp(0, h0, gp0)
            s2f1 = gn_stats_pre(1)          # DVE during scoresT0 (free slot)
            rS0 = softmax_S(0, e0)
            ut0 = ut_gemm(0, h0)            # fills PE while recip0 runs
            mr1 = gn_stats(1, s2f1)         # PE matmuls land after uT0
            h1 = gn_apply(1, mr1, engines="gggg")   # GpSimd, keeps DVE/ACT free
            rsb0 = softmax_bc(0, rS0)
            attn_out_resid(0, ut0, e0, rsb0, x_tiles[0])
            gp1 = gp_gemm(1, h1)
            e1 = scores_exp(1, h1, gp1)
            rS1 = softmax_S(1, e1)
            ut1 = ut_gemm(1, h1)
            rsb1 = softmax_bc(1, rS1)
            attn_out_resid(1, ut1, e1, rsb1, x_tiles[1])

    nc.compile()
    return nc


def _get_nc():
    if "nc" not in _BUILD_CACHE:
        _BUILD_CACHE["nc"] = _build()
    return _BUILD_CACHE["nc"]


def _reference_fallback(x, norm_w, norm_b, qkv_w, qkv_b, proj_w, proj_b):
    """NumPy reference path (only for nonzero qkv/proj biases, which this
    problem's inputs never have)."""
    xf = x.reshape(B, C, N).astype(np.float64)
    xg = xf.reshape(B, G, GS, N)
    mu = xg.mean(axis=(2, 3), keepdims=True)
    var = xg.var(axis=(2, 3), keepdims=True)
    h = ((xg - mu) / np.sqrt(var + EPS)).reshape(B, C, N)
    h = h * norm_w[None, :, None] + norm_b[None, :, None]
    qkv = np.einsum("oc,bcn->bon", qkv_w.astype(np.float64), h) + qkv_b[None, :, None]
    q, k, v = qkv[:, :C], qkv[:, C : 2 * C], qkv[:, 2 * C :]
    z = np.einsum("bci,bcj->bij", q, k) * SCALE
    z -= z.max(axis=2, keepdims=True)
    e = np.exp(z)
    p = e / e.sum(axis=2, keepdims=True)
    o = np.einsum("bcj,bij->bci", v, p)
    out = xf + np.einsum("oc,bcn->bon", proj_w.astype(np.float64), o) + proj_b[None, :, None]
    return out.reshape(B, C, H, W).astype(np.float32)


def _tile_weight(w):
    """[C, C] -> [128, KT*C] with (p, kt*C + c) = w[kt*128+p, c]."""
    return np.ascontiguousarray(
        w.reshape(KT, 128, C).transpose(1, 0, 2).reshape(128, KT * C)
    )


def kernel(x, norm_w, norm_b, qkv_w, qkv_b, proj_w, proj_b, _trace=False):
    global LAST_RESULT

    x = np.asarray(x, dtype=np.float32)
    norm_w = np.asarray(norm_w, dtype=np.float32)
    norm_b = np.asarray(norm_b, dtype=np.float32)
    qkv_w = np.asarray(qkv_w, dtype=np.float32)
    qkv_b = np.asarray(qkv_b, dtype=np.float32)
    proj_w = np.asarray(proj_w, dtype=np.float32)
    proj_b = np.asarray(proj_b, dtype=np.float32)

    if np.any(qkv_b != 0.0) or np.any(proj_b != 0.0):
        return _reference_fallback(
            x, norm_w, norm_b, qkv_w, qkv_b, proj_w, proj_b
        )

    nc = _get_nc()

    # pre-tile x to [B, 128, KT, N] so each sample loads as one descriptor
    xt = np.ascontiguousarray(
        x.reshape(B, KT, 128, N).transpose(0, 2, 1, 3)
    )
    xbf = xt.astype(ml_dtypes.bfloat16)
    Wq = qkv_w[:C].astype(np.float64)
    Wk = qkv_w[C : 2 * C].astype(np.float64)
    Wv = qkv_w[2 * C :].astype(np.float64)
    P = proj_w.astype(np.float64)
    scw = _tile_weight((Wq.T @ Wk).astype(np.float32)).astype(ml_dtypes.bfloat16)
    vw2T = _tile_weight((P @ Wv).T.astype(np.float32)).astype(ml_dtypes.bfloat16)

    ind16 = np.zeros((128, 8), dtype=np.float32)
    for p in range(128):
        ind16[p, p // GS] = 1.0
    ind16T = np.ascontiguousarray(ind16.T)
    ind16 = ind16 * (1.0 / GS)  # fold the group-mean divide into the reduce
    cst17 = np.ascontiguousarray(
        np.concatenate(
            [
                norm_w.reshape(KT, 128).T,
                norm_b.reshape(KT, 128).T,
                ind16,
                np.ones((128, 1), dtype=np.float32),
            ],
            axis=1,
        )
    )
    shared = {
        "scw": scw,
        "vw2T": vw2T,
        "cst17": cst17,
        "ind16T": ind16T,
        "ones_f": np.ones(128, dtype=np.float32),
    }
    in_maps = [
        {
            "xt": np.ascontiguousarray(xt[c * SPC : (c + 1) * SPC]),
            "xbf": np.ascontiguousarray(xbf[c * SPC : (c + 1) * SPC]),
            **shared,
        }
        for c in range(NCORES)
    ]
    res = run_bass_kernel_spmd(nc, in_maps, list(range(NCORES)), trace=_trace)
    LAST_RESULT = res
    out = np.concatenate([res.results[i]["y"] for i in range(NCORES)], axis=0)
    return out.reshape(B, C, H, W)


# revision 47
# speedup vs baseline: 1.0130x; 1.0130x over previous
"""AttnBlock (GroupNorm + 1x1-conv QKV + single-head spatial attention + proj
+ residual) on 8 Trainium2 NeuronCores.

Sharding: pure data-parallel over batch - 16 samples / 8 cores = 2 samples per
core; weights broadcast. No collectives; gather on host.

Algebraic restructuring (exact, cuts PE work ~22% vs the naive formulation):
  scores  = (Wq h)^T (Wk h) = h^T M h   with M  = Wq^T Wk   (host-precomputed)
  y_attn  = P (Wv h) p~     = W2 h p~   with W2 = P Wv      (host-precomputed)
so the kernel runs only 4 GEMM families per sample:
  Gp = M^T h          (C x C x N)      [scores moving operand]
  sT = h^T Gp         (N x N x C)      -> e = exp(s * C^-0.5)  (bf16)
  uT = h^T W2^T       (C x C x N)      [attention value rows]
  o  = uT^T e         (C x N x N)      -> y = x + o * (1/S)
The proj GEMM disappears entirely. qkv_b / proj_b are assumed zero (true for
this problem's inputs; falls back to a NumPy path otherwise); norm_w / norm_b
handled generally.

Internal storage is bf16 (PE streams bf16 at the same 1 col/cycle as fp32r,
but bf16 halves SBUF/DMA). Softmax denominators S via ones-column matmul; 1/S
broadcast to 128 partitions via a rank-1 PE matmul (no DRAM bounce). Warmup
matmuls on a memset tile run during the input-DMA window so the HAM throttle
reaches full clock before real work starts. Input DMAs are kept to 13 large
descriptors (weights pre-tiled host-side, consts merged) so the DMA-semaphore
pool never recycles during the critical startup window; GroupNorm vector ops
are batched across all 4 channel tiles to shorten the stats critical path.
"""

import numpy as np
import ml_dtypes

import concourse.bass as bass
import concourse.tile as tile
from concourse import bacc, mybir
from concourse.bass_utils import run_bass_kernel_spmd

B, C, H, W = 16, 512, 32, 32
N = H * W              # 1024 spatial positions
G = 32                 # groups
GS = C // G            # 16 channels per group
NCORES = 8
SPC = B // NCORES      # samples per core
EPS = 1e-6
SCALE = float(C) ** -0.5
KT = C // 128          # 4 channel tiles of 128
NT = N // 128          # 8 spatial tiles of 128
NH = N // 512          # 2 free-dim halves of 512
NWARM = 42             # PE warmup matmuls (HAM ramp during DMA window)

F32 = mybir.dt.float32
F32R = mybir.dt.float32r
BF16 = mybir.dt.bfloat16

_BUILD_CACHE = {}
LAST_RESULT = None  # BassKernelResults of the most recent run (for test harness)


def _build():
    nc = bacc.Bacc("TRN2", target_bir_lowering=False, debug=False)

    # x pre-tiled host-side to [SPC, 128, KT, N]; f32 for the residual add,
    # bf16 copy for GroupNorm stats+apply (halves the startup-critical DMA)
    xt_ext = nc.declare_dram_parameter("xt", [SPC, 128, KT, N], F32, isOutput=False)
    xbf_ext = nc.declare_dram_parameter("xbf", [SPC, 128, KT, N], BF16, isOutput=False)
    # weights pre-tiled host-side: [128, KT*C] with (p, kt*C + c) = W[kt*128+p, c]
    scw_ext = nc.declare_dram_parameter("scw", [128, KT * C], BF16, isOutput=False)
    vw2T_ext = nc.declare_dram_parameter("vw2T", [128, KT * C], BF16, isOutput=False)
    # merged consts: cols 0-3 norm_w, 4-7 norm_b, 8-15 group indicator, 16 ones
    cst_ext = nc.declare_dram_parameter("cst17", [128, 17], F32, isOutput=False)
    indT_ext = nc.declare_dram_parameter("ind16T", [8, 128], F32, isOutput=False)
    onesf_ext = nc.declare_dram_parameter("ones_f", [128], F32R, isOutput=False)
    y_ext = nc.declare_dram_parameter("y", [SPC, C, N], F32, isOutput=True)

    Identity = mybir.ActivationFunctionType.Identity
    Copy = mybir.ActivationFunctionType.Copy
    Exp = mybir.ActivationFunctionType.Exp
    Sqrt = mybir.ActivationFunctionType.Sqrt
    mult = mybir.AluOpType.mult
    add = mybir.AluOpType.add

    with tile.TileContext(nc) as tc:
        with (
            tc.tile_pool(name="wpool", bufs=1) as wpool,
            tc.tile_pool(name="cpool", bufs=1) as cpool,
            tc.tile_pool(name="xpool", bufs=2) as xpool,
            tc.tile_pool(name="hpool", bufs=2) as hpool,
            tc.tile_pool(name="gpool", bufs=1) as gpool,
            tc.tile_pool(name="upool", bufs=1) as upool,
            tc.tile_pool(name="epool", bufs=1) as epool,
            tc.tile_pool(name="opool", bufs=4) as opool,
            tc.tile_pool(name="gnpool", bufs=2) as gnpool,
            tc.tile_pool(name="spool", bufs=2) as spool,
            tc.tile_pool(name="ps", bufs=8, space="PSUM") as ps,
        ):
            # ---- input DMAs: few large descriptors, split across two queues.
            # Startup-critical: xbf0 (1MB) + weights only; the f32 x copies and
            # sample-1 tensors are gated on h0 (markers below) so xbf0 gets the
            # full DMA bandwidth. ----
            x_tiles = [
                xpool.tile([128, KT, N], F32, tag="x", name=f"x_sb{s}")
                for s in range(SPC)
            ]
            xbf_tiles = [
                xpool.tile([128, KT, N], BF16, tag="xbf", name=f"xbf_sb{s}")
                for s in range(SPC)
            ]
            nc.sync.dma_start(
                out=xbf_tiles[0][:, 0:2, :], in_=xbf_ext.ap()[0][:, 0:2, :]
            )
            nc.gpsimd.dma_start(
                out=xbf_tiles[0][:, 2:KT, :], in_=xbf_ext.ap()[0][:, 2:KT, :]
            )

            cst_sb = cpool.tile([128, 17], F32)
            nc.sync.dma_start(out=cst_sb, in_=cst_ext.ap())
            nw_sb = cst_sb[:, 0:4]
            nb_sb = cst_sb[:, 4:8]
            indT_sb = cpool.tile([8, 128], F32)
            nc.sync.dma_start(out=indT_sb, in_=indT_ext.ap())
            onesf_row = cpool.tile([1, 128], F32R)
            nc.sync.dma_start(out=onesf_row, in_=onesf_ext.ap().unsqueeze(0))

            scw_sb = wpool.tile([128, KT, C], BF16)
            nc.gpsimd.dma_start(
                out=scw_sb, in_=scw_ext.ap()
            )
            vw2T_sb = wpool.tile([128, KT, C], BF16)
            # gated (marker-dependent) loads, emitted later in the schedule

            # small derived consts (DVE casts off the merged tile)
            ind_sb = cst_sb[:, 8:16]
            onesb_col = cpool.tile([128, 1], BF16)
            nc.vector.tensor_copy(onesb_col, cst_sb[:, 16:17])

            # ---- ACT table warmups ----
            eps_sb = cpool.tile([8, 1], F32)
            nc.vector.memset(eps_sb, EPS)
            warm_sb = cpool.tile([8, 1], F32)
            nc.scalar.activation(out=warm_sb, in_=eps_sb, func=Identity)
            nc.scalar.activation(out=warm_sb, in_=eps_sb, func=Sqrt)
            nc.scalar.activation(out=warm_sb, in_=eps_sb, func=Exp)

            # ---- PE warmup: no-dependency matmuls on a memset tile so the
            # HAM throttle ramps to full clock while x0 is still loading.
            # Emitted in chunks interleaved with the GroupNorm matmuls so the
            # PE stays busy through the gn small-op chain. ----
            warm_f = cpool.tile([128, 512], F32)
            nc.vector.memset(warm_f, 0.25)
            warm_mm = cpool.tile([128, 512], BF16)
            nc.vector.tensor_copy(warm_mm, warm_f)

            def warm(n):
                for _ in range(n):
                    pw = ps.tile([128, 512], F32, tag="mm")
                    nc.tensor.matmul(
                        pw, warm_mm[:, 0:128], warm_mm, start=True, stop=True
                    )

            def gn_stats_pre(s):
                """Per-partition moment DVE work for sample s -> s2f tile.

                All cross-kt vector work is batched into single strided ops to
                keep the serial small-op chain short."""
                x_sb = xbf_tiles[s]
                s2f = gnpool.tile([128, KT, 2], F32, tag="s2f", name=f"s2f{s}")
                for kt in range(KT):
                    stats = gnpool.tile(
                        [128, 2, 6], F32, tag=f"stats{kt}", name=f"stats{s}_{kt}"
                    )
                    for sg in range(2):
                        nc.vector.bn_stats(
                            out=stats[:, sg, :],
                            in_=x_sb[:, kt, sg * 512 : (sg + 1) * 512],
                        )
                    nc.vector.bn_aggr(out=s2f[:, kt, :], in_=stats)
                # turn [mean, var] into [mean, E[x^2]] in place, batched
                msq2 = gnpool.tile([128, KT], F32, tag="msq2", name=f"msq2{s}")
                nc.vector.tensor_mul(msq2, s2f[:, :, 0], s2f[:, :, 0])
                nc.vector.tensor_add(s2f[:, :, 1], s2f[:, :, 1], msq2)
                return s2f

            def gn_stats(s, s2f=None):
                """GroupNorm stats -> mr (8,KT,2) F32 [group mean, rstd]."""
                if s2f is None:
                    s2f = gn_stats_pre(s)
                # ind_sb carries 1/16 so ps_gs is already the group mean
                ps_gs = ps.tile([8, KT, 2], F32, tag="mm", name=f"ps_gs{s}")
                nc.tensor.matmul(ps_gs, ind_sb, s2f, start=True, stop=True)
                mr = gnpool.tile([8, KT, 2], F32, tag="mr", name=f"mr{s}")
                msq = gnpool.tile([8, KT], F32, tag="msq", name=f"msq{s}")
                nc.vector.tensor_copy(mr[:, :, 0], ps_gs[:, :, 0])
                nc.vector.tensor_mul(msq, mr[:, :, 0], mr[:, :, 0])
                nc.vector.tensor_sub(mr[:, :, 1], ps_gs[:, :, 1], msq)
                nc.scalar.activation(
                    out=mr[:, :, 1], in_=mr[:, :, 1], func=Sqrt, bias=eps_sb
                )
                nc.vector.reciprocal(mr[:, :, 1], mr[:, :, 1])
                return mr

            def gn_apply(s, mr, engines):
                """Broadcast stats to channels and apply x*scale+bias -> h bf16.

                engines: per-kt 'v' (DVE), 'a' (ACT) or 'g' (GpSimd)."""
                x_sb = xbf_tiles[s]
                h_sb = hpool.tile([128, KT, N], BF16, tag="h", name=f"h{s}")
                ps_bc = ps.tile([128, KT, 2], F32, tag="mm", name=f"ps_bc{s}")
                nc.tensor.matmul(ps_bc, indT_sb, mr, start=True, stop=True)
                scb = gnpool.tile([128, KT, 2], F32, tag="scb", name=f"scb{s}")
                nc.vector.tensor_mul(scb[:, :, 0], ps_bc[:, :, 1], nw_sb)
                nc.vector.tensor_mul(scb[:, :, 1], ps_bc[:, :, 0], scb[:, :, 0])
                nc.vector.tensor_sub(scb[:, :, 1], nb_sb, scb[:, :, 1])
                for kt in range(KT):
                    args = dict(
                        out=h_sb[:, kt, :],
                        in0=x_sb[:, kt, :],
                        scalar1=scb[:, kt, 0:1],
                        scalar2=scb[:, kt, 1:2],
                        op0=mult,
                        op1=add,
                    )
                    if engines[kt] == "v":
                        nc.vector.tensor_scalar(**args)
                    elif engines[kt] == "g":
                        nc.gpsimd.tensor_scalar(**args)
                    else:
                        nc.scalar.activation(
                            out=h_sb[:, kt, :], in_=x_sb[:, kt, :],
                            func=Identity, scale=scb[:, kt, 0:1],
                            bias=scb[:, kt, 1:2],
                        )
                return h_sb

            def gp_gemm(s, h_sb):
                """Gp = M^T h -> [128(c'), KT, N] bf16. Moves split DVE/ACT."""
                gp_sb = gpool.tile([128, KT, N], BF16, tag="gp", name=f"gp{s}")
                for obl in range(KT):
                    for ih in range(NH):
                        pm = ps.tile([128, 512], F32, tag="mm")
                        for kt in range(KT):
                            nc.tensor.matmul(
                                pm,
                                scw_sb[:, kt, obl * 128 : (obl + 1) * 128],
                                h_sb[:, kt, ih * 512 : (ih + 1) * 512],
                                start=(kt == 0),
                                stop=(kt == KT - 1),
                            )
                        if obl % 2 == 0:
                            nc.vector.tensor_copy(
                                gp_sb[:, obl, ih * 512 : (ih + 1) * 512], pm
                            )
                        else:
                            nc.scalar.activation(
                                out=gp_sb[:, obl, ih * 512 : (ih + 1) * 512],
                                in_=pm, func=Identity,
                            )
                return gp_sb

            def scores_exp(s, h_sb, gp_sb):
                """e = exp(scale * h^T Gp) -> [128(j), NT, N(i)] bf16 (ACT)."""
                e_sb = epool.tile([128, NT, N], BF16, tag="e", name=f"e{s}")
                for jt in range(NT):
                    for ih in range(NH):
                        pm = ps.tile([128, 512], F32, tag="mm")
                        for kt in range(KT):
                            nc.tensor.matmul(
                                pm,
                                h_sb[:, kt, jt * 128 : (jt + 1) * 128],
                                gp_sb[:, kt, ih * 512 : (ih + 1) * 512],
                                start=(kt == 0),
                                stop=(kt == KT - 1),
                            )
                        nc.scalar.activation(
                            out=e_sb[:, jt, ih * 512 : (ih + 1) * 512],
                            in_=pm,
                            func=Exp,
                            scale=SCALE,
                        )
                return e_sb

            def ut_gemm(s, h_sb):
                """uT = h^T W2^T -> [128(j), NT, C] bf16. PSUM moves on ACT."""
                ut_sb = upool.tile([128, NT, C], BF16, tag="ut", name=f"ut{s}")
                for nt in range(NT):
                    pm = ps.tile([128, 512], F32, tag="mm")
                    for kt in range(KT):
                        nc.tensor.matmul(
                            pm,
                            h_sb[:, kt, nt * 128 : (nt + 1) * 128],
                            vw2T_sb[:, kt, :],
                            start=(kt == 0),
                            stop=(kt == KT - 1),
                        )
                    nc.scalar.activation(
                        out=ut_sb[:, nt, :], in_=pm, func=Identity
                    )
                return ut_sb

            def softmax_S(s, e_sb):
                """S = ones^T e -> 1/S on one partition (DVE recip)."""
                recipSf = spool.tile([1, N], F32, tag="recipSf", name=f"recipSf{s}")
                recipS = spool.tile([1, N], F32R, tag="recipS", name=f"recipS{s}")
                for ih in range(NH):
                    pS = ps.tile([1, 512], F32, tag="mm")
                    for jt in range(NT):
                        nc.tensor.matmul(
                            pS,
                            onesb_col,
                            e_sb[:, jt, ih * 512 : (ih + 1) * 512],
                            start=(jt == 0),
                            stop=(jt == NT - 1),
                        )
                    nc.vector.reciprocal_approx_fast(
                        out=recipSf[:, ih * 512 : (ih + 1) * 512], in_=pS
                    )
                nc.vector.tensor_copy(recipS, recipSf)
                return recipS

            def softmax_bc(s, recipS):
                """1/S broadcast to 128 partitions via PE rank-1 matmul
                -> rsb [128, N] f32 in SBUF (ACT copies)."""
                rsb = spool.tile([128, N], F32, tag="rsb", name=f"rsb{s}")
                for ih in range(NH):
                    pbc = ps.tile([128, 512], F32, tag="mm")
                    nc.tensor.matmul(
                        pbc,
                        onesf_row,
                        recipS[:, ih * 512 : (ih + 1) * 512],
                        start=True,
                        stop=True,
                    )
                    nc.scalar.activation(
                        out=rsb[:, ih * 512 : (ih + 1) * 512], in_=pbc,
                        func=Identity,
                    )
                return rsb

            def attn_out_resid(s, ut_sb, e_sb, rsb, x_sb):
                """y = x + (uT^T e) * rS, chunkwise: DVE mult+add, y DMAs
                alternate between the gpsimd and sync queues. The last chunk
                of the last sample is processed in half-width pieces so the
                post-matmul trail (mult+add+DMA) is shorter."""
                for ct in range(KT):
                    for ih in range(NH):
                        pm = ps.tile([128, 512], F32, tag="mm")
                        for jt in range(NT):
                            nc.tensor.matmul(
                                pm,
                                ut_sb[:, jt, ct * 128 : (ct + 1) * 128],
                                e_sb[:, jt, ih * 512 : (ih + 1) * 512],
                                start=(jt == 0),
                                stop=(jt == NT - 1),
                            )
                        oy = opool.tile(
                            [128, 512], F32, tag="oy", name=f"oy{s}_{ct}_{ih}"
                        )
                        last = s == SPC - 1 and ct == KT - 1 and ih == NH - 1
                        pieces = 2 if last else 1
                        pw = 512 // pieces
                        for pc in range(pieces):
                            pslc = slice(pc * pw, (pc + 1) * pw)
                            yslc = slice(
                                ih * 512 + pc * pw, ih * 512 + (pc + 1) * pw
                            )
                            nc.vector.tensor_mul(
                                oy[:, pslc], pm[:, pslc], rsb[:, yslc]
                            )
                            xc = x_sb[:, ct, yslc]
                            nc.vector.tensor_add(xc, oy[:, pslc], xc)
                            dma_eng = (
                                nc.gpsimd
                                if (ct * NH + ih + pc) % 2 == 0
                                else nc.sync
                            )
                            dma_eng.dma_start(
                                out=y_ext.ap()[
                                    s, ct * 128 : (ct + 1) * 128, yslc
                                ],
                                in_=xc,
                            )

            # ---- schedule: two samples, phases interleaved ----
            warm(30)
            mr0 = gn_stats(0)
            warm(6)
            h0 = gn_apply(0, mr0, engines="vagv")
            warm(6)
            # Remaining transfers (f32 x, sample-1 bf16 x, vw2T) gated on h0
            # via tiny marker writes, so xbf0 gets full HBM bandwidth during
            # the startup-critical window (sample-0 GroupNorm gates all PE
            # work). Marker values are overwritten by the DMAs. Order matters:
            # xbf1 first (bn1 needs it ~35us), then vw2T (~45us), then f32 x
            # (residual adds, ~60us / ~105us).
            for t in (x_tiles[0], x_tiles[1], xbf_tiles[1], vw2T_sb):
                nc.gpsimd.tensor_copy(t[0:1, 0, 0:1], h0[0:1, 0, 0:1])
            nc.gpsimd.tensor_copy(xbf_tiles[1][0:1, 2, 0:1], h0[0:1, 0, 0:1])
            nc.sync.dma_start(
                out=xbf_tiles[1][:, 0:2, :], in_=xbf_ext.ap()[1][:, 0:2, :]
            )
            nc.gpsimd.dma_start(
                out=xbf_tiles[1][:, 2:KT, :], in_=xbf_ext.ap()[1][:, 2:KT, :]
            )
            nc.gpsimd.dma_start(out=vw2T_sb, in_=vw2T_ext.ap())
            nc.sync.dma_start(out=x_tiles[0], in_=xt_ext.ap()[0])
            nc.gpsimd.dma_start(out=x_tiles[1], in_=xt_ext.ap()[1])
            gp0 = gp_gemm(0, h0)
            e0 = scores_exp(0, h0, gp0)
            s2f1 = gn_stats_pre(1)          # DVE during scoresT0 (free slot)
            rS0 = softmax_S(0, e0)
            ut0 = ut_gemm(0, h0)            # fills PE while recip0 runs
            mr1 = gn_stats(1, s2f1)         # PE matmuls land after uT0
            h1 = gn_apply(1, mr1, engines="gggg")   # GpSimd, keeps DVE/ACT free
            rsb0 = softmax_bc(0, rS0)
            attn_out_resid(0, ut0, e0, rsb0, x_tiles[0])
            gp1 = gp_gemm(1, h1)
            e1 = scores_exp(1, h1, gp1)
            rS1 = softmax_S(1, e1)
            ut1 = ut_gemm(1, h1)
            rsb1 = softmax_bc(1, rS1)
            attn_out_resid(1, ut1, e1, rsb1, x_tiles[1])

    nc.compile()
    return nc


def _get_nc():
    if "nc" not in _BUILD_CACHE:
        _BUILD_CACHE["nc"] = _build()
    return _BUILD_CACHE["nc"]


def _reference_fallback(x, norm_w, norm_b, qkv_w, qkv_b, proj_w, proj_b):
    """NumPy reference path (only for nonzero qkv/proj biases, which this
    problem's inputs never have)."""
    xf = x.reshape(B, C, N).astype(np.float64)
    xg = xf.reshape(B, G, GS, N)
    mu = xg.mean(axis=(2, 3), keepdims=True)
    var = xg.var(axis=(2, 3), keepdims=True)
    h = ((xg - mu) / np.sqrt(var + EPS)).reshape(B, C, N)
    h = h * norm_w[None, :, None] + norm_b[None, :, None]
    qkv = np.einsum("oc,bcn->bon", qkv_w.astype(np.float64), h) + qkv_b[None, :, None]
    q, k, v = qkv[:, :C], qkv[:, C : 2 * C], qkv[:, 2 * C :]
    z = np.einsum("bci,bcj->bij", q, k) * SCALE
    z -= z.max(axis=2, keepdims=True)
    e = np.exp(z)
    p = e / e.sum(axis=2, keepdims=True)
    o = np.einsum("bcj,bij->bci", v, p)
    out = xf + np.einsum("oc,bcn->bon", proj_w.astype(np.float64), o) + proj_b[None, :, None]
    return out.reshape(B, C, H, W).astype(np.float32)


def _tile_weight(w):
    """[C, C] -> [128, KT*C] with (p, kt*C + c) = w[kt*128+p, c]."""
    return np.ascontiguousarray(
        w.reshape(KT, 128, C).transpose(1, 0, 2).reshape(128, KT * C)
    )


def kernel(x, norm_w, norm_b, qkv_w, qkv_b, proj_w, proj_b, _trace=False):
    global LAST_RESULT

    x = np.asarray(x, dtype=np.float32)
    norm_w = np.asarray(norm_w, dtype=np.float32)
    norm_b = np.asarray(norm_b, dtype=np.float32)
    qkv_w = np.asarray(qkv_w, dtype=np.float32)
    qkv_b = np.asarray(qkv_b, dtype=np.float32)
    proj_w = np.asarray(proj_w, dtype=np.float32)
    proj_b = np.asarray(proj_b, dtype=np.float32)

    if np.any(qkv_b != 0.0) or np.any(proj_b != 0.0):
        return _reference_fallback(
            x, norm_w, norm_b, qkv_w, qkv_b, proj_w, proj_b
        )

    nc = _get_nc()

    # pre-tile x to [B, 128, KT, N] so each sample loads as one descriptor
    xt = np.ascontiguousarray(
        x.reshape(B, KT, 128, N).transpose(0, 2, 1, 3)
    )
    xbf = xt.astype(ml_dtypes.bfloat16)
    Wq = qkv_w[:C].astype(np.float64)
    Wk = qkv_w[C : 2 * C].astype(np.float64)
    Wv = qkv_w[2 * C :].astype(np.float64)
    P = proj_w.astype(np.float64)
    scw = _tile_weight((Wq.T @ Wk).astype(np.float32)).astype(ml_dtypes.bfloat16)
    vw2T = _tile_weight((P @ Wv).T.astype(np.float32)).astype(ml_dtypes.bfloat16)

    ind16 = np.zeros((128, 8), dtype=np.float32)
    for p in range(128):
        ind16[p, p // GS] = 1.0
    ind16T = np.ascontiguousarray(ind16.T)
    ind16 = ind16 * (1.0 / GS)  # fold the group-mean divide into the reduce
    cst17 = np.ascontiguousarray(
        np.concatenate(
            [
                norm_w.reshape(KT, 128).T,
                norm_b.reshape(KT, 128).T,
                ind16,
                np.ones((128, 1), dtype=np.float32),
            ],
            axis=1,
        )
    )
    shared = {
        "scw": scw,
        "vw2T": vw2T,
        "cst17": cst17,
        "ind16T": ind16T,
        "ones_f": np.ones(128, dtype=np.float32),
    }
    in_maps = [
        {
            "xt": np.ascontiguousarray(xt[c * SPC : (c + 1) * SPC]),
            "xbf": np.ascontiguousarray(xbf[c * SPC : (c + 1) * SPC]),
            **shared,
        }
        for c in range(NCORES)
    ]
    res = run_bass_kernel_spmd(nc, in_maps, list(range(NCORES)), trace=_trace)
    LAST_RESULT = res
    out = np.concatenate([res.results[i]["y"] for i in range(NCORES)], axis=0)
    return out.reshape(B, C, H, W)


# revision 48
# speedup vs baseline: 1.0149x; 1.0018x over previous
"""AttnBlock (GroupNorm + 1x1-conv QKV + single-head spatial attention + proj
+ residual) on 8 Trainium2 NeuronCores.

Sharding: pure data-parallel over batch - 16 samples / 8 cores = 2 samples per
core; weights broadcast. No collectives; gather on host.

Algebraic restructuring (exact, cuts PE work ~22% vs the naive formulation):
  scores  = (Wq h)^T (Wk h) = h^T M h   with M  = Wq^T Wk   (host-precomputed)
  y_attn  = P (Wv h) p~     = W2 h p~   with W2 = P Wv      (host-precomputed)
so the kernel runs only 4 GEMM families per sample:
  Gp = M^T h          (C x C x N)      [scores moving operand]
  sT = h^T Gp         (N x N x C)      -> e = exp(s * C^-0.5)  (bf16)
  uT = h^T W2^T       (C x C x N)      [attention value rows]
  o  = uT^T e         (C x N x N)      -> y = x + o * (1/S)
The proj GEMM disappears entirely. qkv_b / proj_b are assumed zero (true for
this problem's inputs; falls back to a NumPy path otherwise); norm_w / norm_b
handled generally.

Internal storage is bf16 (PE streams bf16 at the same 1 col/cycle as fp32r,
but bf16 halves SBUF/DMA). Softmax denominators S via ones-column matmul; 1/S
broadcast to 128 partitions via a rank-1 PE matmul (no DRAM bounce). Warmup
matmuls on a memset tile run during the input-DMA window so the HAM throttle
reaches full clock before real work starts. Input DMAs are kept to 13 large
descriptors (weights pre-tiled host-side, consts merged) so the DMA-semaphore
pool never recycles during the critical startup window; GroupNorm vector ops
are batched across all 4 channel tiles to shorten the stats critical path.
"""

import numpy as np
import ml_dtypes

import concourse.bass as bass
import concourse.tile as tile
from concourse import bacc, mybir
from concourse.bass_utils import run_bass_kernel_spmd

B, C, H, W = 16, 512, 32, 32
N = H * W              # 1024 spatial positions
G = 32                 # groups
GS = C // G            # 16 channels per group
NCORES = 8
SPC = B // NCORES      # samples per core
EPS = 1e-6
SCALE = float(C) ** -0.5
KT = C // 128          # 4 channel tiles of 128
NT = N // 128          # 8 spatial tiles of 128
NH = N // 512          # 2 free-dim halves of 512
NWARM = 47             # PE warmup matmuls (HAM ramp during DMA window)

F32 = mybir.dt.float32
F32R = mybir.dt.float32r
BF16 = mybir.dt.bfloat16

_BUILD_CACHE = {}
LAST_RESULT = None  # BassKernelResults of the most recent run (for test harness)


def _build():
    nc = bacc.Bacc("TRN2", target_bir_lowering=False, debug=False)

    # x pre-tiled host-side to [SPC, 128, KT, N]; f32 for the residual add,
    # bf16 copy for GroupNorm stats+apply (halves the startup-critical DMA)
    xt_ext = nc.declare_dram_parameter("xt", [SPC, 128, KT, N], F32, isOutput=False)
    xbf_ext = nc.declare_dram_parameter("xbf", [SPC, 128, KT, N], BF16, isOutput=False)
    # weights pre-tiled host-side: [128, KT*C] with (p, kt*C + c) = W[kt*128+p, c]
    scw_ext = nc.declare_dram_parameter("scw", [128, KT * C], BF16, isOutput=False)
    vw2T_ext = nc.declare_dram_parameter("vw2T", [128, KT * C], BF16, isOutput=False)
    # merged consts: cols 0-3 norm_w, 4-7 norm_b, 8-15 group indicator, 16 ones
    cst_ext = nc.declare_dram_parameter("cst17", [128, 17], F32, isOutput=False)
    indT_ext = nc.declare_dram_parameter("ind16T", [8, 128], F32, isOutput=False)
    onesf_ext = nc.declare_dram_parameter("ones_f", [128], F32R, isOutput=False)
    y_ext = nc.declare_dram_parameter("y", [SPC, C, N], F32, isOutput=True)

    Identity = mybir.ActivationFunctionType.Identity
    Copy = mybir.ActivationFunctionType.Copy
    Exp = mybir.ActivationFunctionType.Exp
    Sqrt = mybir.ActivationFunctionType.Sqrt
    mult = mybir.AluOpType.mult
    add = mybir.AluOpType.add

    with tile.TileContext(nc) as tc:
        with (
            tc.tile_pool(name="wpool", bufs=1) as wpool,
            tc.tile_pool(name="cpool", bufs=1) as cpool,
            tc.tile_pool(name="xpool", bufs=2) as xpool,
            tc.tile_pool(name="hpool", bufs=2) as hpool,
            tc.tile_pool(name="gpool", bufs=1) as gpool,
            tc.tile_pool(name="upool", bufs=1) as upool,
            tc.tile_pool(name="epool", bufs=1) as epool,
            tc.tile_pool(name="opool", bufs=4) as opool,
            tc.tile_pool(name="gnpool", bufs=2) as gnpool,
            tc.tile_pool(name="spool", bufs=2) as spool,
            tc.tile_pool(name="ps", bufs=8, space="PSUM") as ps,
        ):
            # ---- input DMAs: few large descriptors, split across two queues.
            # Startup-critical: xbf0 (1MB) + weights only; the f32 x copies and
            # sample-1 tensors are gated on h0 (markers below) so xbf0 gets the
            # full DMA bandwidth. ----
            x_tiles = [
                xpool.tile([128, KT, N], F32, tag="x", name=f"x_sb{s}")
                for s in range(SPC)
            ]
            xbf_tiles = [
                xpool.tile([128, KT, N], BF16, tag="xbf", name=f"xbf_sb{s}")
                for s in range(SPC)
            ]
            nc.sync.dma_start(
                out=xbf_tiles[0][:, 0:2, :], in_=xbf_ext.ap()[0][:, 0:2, :]
            )
            nc.gpsimd.dma_start(
                out=xbf_tiles[0][:, 2:KT, :], in_=xbf_ext.ap()[0][:, 2:KT, :]
            )

            cst_sb = cpool.tile([128, 17], F32)
            nc.sync.dma_start(out=cst_sb, in_=cst_ext.ap())
            nw_sb = cst_sb[:, 0:4]
            nb_sb = cst_sb[:, 4:8]
            indT_sb = cpool.tile([8, 128], F32)
            nc.sync.dma_start(out=indT_sb, in_=indT_ext.ap())
            onesf_row = cpool.tile([1, 128], F32R)
            nc.sync.dma_start(out=onesf_row, in_=onesf_ext.ap().unsqueeze(0))

            scw_sb = wpool.tile([128, KT, C], BF16)
            nc.gpsimd.dma_start(
                out=scw_sb, in_=scw_ext.ap()
            )
            vw2T_sb = wpool.tile([128, KT, C], BF16)
            # gated (marker-dependent) loads, emitted later in the schedule

            # small derived consts (DVE casts off the merged tile)
            ind_sb = cst_sb[:, 8:16]
            onesb_col = cpool.tile([128, 1], BF16)
            nc.vector.tensor_copy(onesb_col, cst_sb[:, 16:17])

            # ---- ACT table warmups ----
            eps_sb = cpool.tile([8, 1], F32)
            nc.vector.memset(eps_sb, EPS)
            warm_sb = cpool.tile([8, 1], F32)
            nc.scalar.activation(out=warm_sb, in_=eps_sb, func=Identity)
            nc.scalar.activation(out=warm_sb, in_=eps_sb, func=Sqrt)
            nc.scalar.activation(out=warm_sb, in_=eps_sb, func=Exp)

            # ---- PE warmup: no-dependency matmuls on a memset tile so the
            # HAM throttle ramps to full clock while x0 is still loading.
            # Emitted in chunks interleaved with the GroupNorm matmuls so the
            # PE stays busy through the gn small-op chain. ----
            warm_f = cpool.tile([128, 512], F32)
            nc.vector.memset(warm_f, 0.25)
            warm_mm = cpool.tile([128, 512], BF16)
            nc.vector.tensor_copy(warm_mm, warm_f)

            def warm(n):
                for _ in range(n):
                    pw = ps.tile([128, 512], F32, tag="mm")
                    nc.tensor.matmul(
                        pw, warm_mm[:, 0:128], warm_mm, start=True, stop=True
                    )

            def gn_stats_pre(s):
                """Per-partition moment DVE work for sample s -> s2f tile.

                All cross-kt vector work is batched into single strided ops to
                keep the serial small-op chain short."""
                x_sb = xbf_tiles[s]
                s2f = gnpool.tile([128, KT, 2], F32, tag="s2f", name=f"s2f{s}")
                for kt in range(KT):
                    stats = gnpool.tile(
                        [128, 2, 6], F32, tag=f"stats{kt}", name=f"stats{s}_{kt}"
                    )
                    for sg in range(2):
                        nc.vector.bn_stats(
                            out=stats[:, sg, :],
                            in_=x_sb[:, kt, sg * 512 : (sg + 1) * 512],
                        )
                    nc.vector.bn_aggr(out=s2f[:, kt, :], in_=stats)
                # turn [mean, var] into [mean, E[x^2]] in place, batched
                msq2 = gnpool.tile([128, KT], F32, tag="msq2", name=f"msq2{s}")
                nc.vector.tensor_mul(msq2, s2f[:, :, 0], s2f[:, :, 0])
                nc.vector.tensor_add(s2f[:, :, 1], s2f[:, :, 1], msq2)
                return s2f

            def gn_stats(s, s2f=None):
                """GroupNorm stats -> mr (8,KT,2) F32 [group mean, rstd]."""
                if s2f is None:
                    s2f = gn_stats_pre(s)
                # ind_sb carries 1/16 so ps_gs is already the group mean
                ps_gs = ps.tile([8, KT, 2], F32, tag="mm", name=f"ps_gs{s}")
                nc.tensor.matmul(ps_gs, ind_sb, s2f, start=True, stop=True)
                mr = gnpool.tile([8, KT, 2], F32, tag="mr", name=f"mr{s}")
                msq = gnpool.tile([8, KT], F32, tag="msq", name=f"msq{s}")
                nc.vector.tensor_copy(mr[:, :, 0], ps_gs[:, :, 0])
                nc.vector.tensor_mul(msq, mr[:, :, 0], mr[:, :, 0])
                nc.vector.tensor_sub(mr[:, :, 1], ps_gs[:, :, 1], msq)
                nc.scalar.activation(
                    out=mr[:, :, 1], in_=mr[:, :, 1], func=Sqrt, bias=eps_sb
                )
                nc.vector.reciprocal(mr[:, :, 1], mr[:, :, 1])
                return mr

            def gn_apply(s, mr, engines):
                """Broadcast stats to channels and apply x*scale+bias -> h bf16.

                engines: per-kt 'v' (DVE), 'a' (ACT) or 'g' (GpSimd)."""
                x_sb = xbf_tiles[s]
                h_sb = hpool.tile([128, KT, N], BF16, tag="h", name=f"h{s}")
                ps_bc = ps.tile([128, KT, 2], F32, tag="mm", name=f"ps_bc{s}")
                nc.tensor.matmul(ps_bc, indT_sb, mr, start=True, stop=True)
                scb = gnpool.tile([128, KT, 2], F32, tag="scb", name=f"scb{s}")
                nc.vector.tensor_mul(scb[:, :, 0], ps_bc[:, :, 1], nw_sb)
                nc.vector.tensor_mul(scb[:, :, 1], ps_bc[:, :, 0], scb[:, :, 0])
                nc.vector.tensor_sub(scb[:, :, 1], nb_sb, scb[:, :, 1])
                for kt in range(KT):
                    args = dict(
                        out=h_sb[:, kt, :],
                        in0=x_sb[:, kt, :],
                        scalar1=scb[:, kt, 0:1],
                        scalar2=scb[:, kt, 1:2],
                        op0=mult,
                        op1=add,
                    )
                    if engines[kt] == "v":
                        nc.vector.tensor_scalar(**args)
                    elif engines[kt] == "g":
                        nc.gpsimd.tensor_scalar(**args)
                    else:
                        nc.scalar.activation(
                            out=h_sb[:, kt, :], in_=x_sb[:, kt, :],
                            func=Identity, scale=scb[:, kt, 0:1],
                            bias=scb[:, kt, 1:2],
                        )
                return h_sb

            def gp_gemm(s, h_sb):
                """Gp = M^T h -> [128(c'), KT, N] bf16. Moves split DVE/ACT."""
                gp_sb = gpool.tile([128, KT, N], BF16, tag="gp", name=f"gp{s}")
                for obl in range(KT):
                    for ih in range(NH):
                        pm = ps.tile([128, 512], F32, tag="mm")
                        for kt in range(KT):
                            nc.tensor.matmul(
                                pm,
                                scw_sb[:, kt, obl * 128 : (obl + 1) * 128],
                                h_sb[:, kt, ih * 512 : (ih + 1) * 512],
                                start=(kt == 0),
                                stop=(kt == KT - 1),
                            )
                        if obl % 2 == 0:
                            nc.vector.tensor_copy(
                                gp_sb[:, obl, ih * 512 : (ih + 1) * 512], pm
                            )
                        else:
                            nc.scalar.activation(
                                out=gp_sb[:, obl, ih * 512 : (ih + 1) * 512],
                                in_=pm, func=Identity,
                            )
                return gp_sb

            def scores_exp(s, h_sb, gp_sb):
                """e = exp(scale * h^T Gp) -> [128(j), NT, N(i)] bf16 (ACT)."""
                e_sb = epool.tile([128, NT, N], BF16, tag="e", name=f"e{s}")
                for jt in range(NT):
                    for ih in range(NH):
                        pm = ps.tile([128, 512], F32, tag="mm")
                        for kt in range(KT):
                            nc.tensor.matmul(
                                pm,
                                h_sb[:, kt, jt * 128 : (jt + 1) * 128],
                                gp_sb[:, kt, ih * 512 : (ih + 1) * 512],
                                start=(kt == 0),
                                stop=(kt == KT - 1),
                            )
                        nc.scalar.activation(
                            out=e_sb[:, jt, ih * 512 : (ih + 1) * 512],
                            in_=pm,
                            func=Exp,
                            scale=SCALE,
                        )
                return e_sb

            def ut_gemm(s, h_sb):
                """uT = h^T W2^T -> [128(j), NT, C] bf16. PSUM moves on ACT."""
                ut_sb = upool.tile([128, NT, C], BF16, tag="ut", name=f"ut{s}")
                for nt in range(NT):
                    pm = ps.tile([128, 512], F32, tag="mm")
                    for kt in range(KT):
                        nc.tensor.matmul(
                            pm,
                            h_sb[:, kt, nt * 128 : (nt + 1) * 128],
                            vw2T_sb[:, kt, :],
                            start=(kt == 0),
                            stop=(kt == KT - 1),
                        )
                    nc.scalar.activation(
                        out=ut_sb[:, nt, :], in_=pm, func=Identity
                    )
                return ut_sb

            def softmax_S(s, e_sb):
                """S = ones^T e -> 1/S on one partition (DVE recip)."""
                recipSf = spool.tile([1, N], F32, tag="recipSf", name=f"recipSf{s}")
                recipS = spool.tile([1, N], F32R, tag="recipS", name=f"recipS{s}")
                for ih in range(NH):
                    pS = ps.tile([1, 512], F32, tag="mm")
                    for jt in range(NT):
                        nc.tensor.matmul(
                            pS,
                            onesb_col,
                            e_sb[:, jt, ih * 512 : (ih + 1) * 512],
                            start=(jt == 0),
                            stop=(jt == NT - 1),
                        )
                    nc.vector.reciprocal_approx_fast(
                        out=recipSf[:, ih * 512 : (ih + 1) * 512], in_=pS
                    )
                nc.vector.tensor_copy(recipS, recipSf)
                return recipS

            def softmax_bc(s, recipS):
                """1/S broadcast to 128 partitions via PE rank-1 matmul
                -> rsb [128, N] f32 in SBUF (ACT copies)."""
                rsb = spool.tile([128, N], F32, tag="rsb", name=f"rsb{s}")
                for ih in range(NH):
                    pbc = ps.tile([128, 512], F32, tag="mm")
                    nc.tensor.matmul(
                        pbc,
                        onesf_row,
                        recipS[:, ih * 512 : (ih + 1) * 512],
                        start=True,
                        stop=True,
                    )
                    nc.scalar.activation(
                        out=rsb[:, ih * 512 : (ih + 1) * 512], in_=pbc,
                        func=Identity,
                    )
                return rsb

            def attn_out_resid(s, ut_sb, e_sb, rsb, x_sb):
                """y = x + (uT^T e) * rS, chunkwise: DVE mult+add, y DMAs
                alternate between the gpsimd and sync queues. The last chunk
                of the last sample is processed in half-width pieces so the
                post-matmul trail (mult+add+DMA) is shorter."""
                for ct in range(KT):
                    for ih in range(NH):
                        pm = ps.tile([128, 512], F32, tag="mm")
                        for jt in range(NT):
                            nc.tensor.matmul(
                                pm,
                                ut_sb[:, jt, ct * 128 : (ct + 1) * 128],
                                e_sb[:, jt, ih * 512 : (ih + 1) * 512],
                                start=(jt == 0),
                                stop=(jt == NT - 1),
                            )
                        oy = opool.tile(
                            [128, 512], F32, tag="oy", name=f"oy{s}_{ct}_{ih}"
                        )
                        last = s == SPC - 1 and ct == KT - 1 and ih == NH - 1
                        pieces = 2 if last else 1
                        pw = 512 // pieces
                        for pc in range(pieces):
                            pslc = slice(pc * pw, (pc + 1) * pw)
                            yslc = slice(
                                ih * 512 + pc * pw, ih * 512 + (pc + 1) * pw
                            )
                            nc.vector.tensor_mul(
                                oy[:, pslc], pm[:, pslc], rsb[:, yslc]
                            )
                            xc = x_sb[:, ct, yslc]
                            nc.vector.tensor_add(xc, oy[:, pslc], xc)
                            dma_eng = (
                                nc.gpsimd
                                if (ct * NH + ih + pc) % 2 == 0
                                else nc.sync
                            )
                            dma_eng.dma_start(
                                out=y_ext.ap()[
                                    s, ct * 128 : (ct + 1) * 128, yslc
                                ],
                                in_=xc,
                            )

            # ---- schedule: two samples, phases interleaved ----
            warm(30)
            mr0 = gn_stats(0)
            warm(6)
            h0 = gn_apply(0, mr0, engines="vagv")
            warm(6)
            # Remaining transfers (f32 x, sample-1 bf16 x, vw2T) gated on h0
            # via tiny marker writes, so xbf0 gets full HBM bandwidth during
            # the startup-critical window (sample-0 GroupNorm gates all PE
            # work). Marker values are overwritten by the DMAs. Order matters:
            # xbf1 first (bn1 needs it ~35us), then vw2T (~45us), then f32 x
            # (residual adds, ~60us / ~105us).
            for t in (x_tiles[0], x_tiles[1], xbf_tiles[1], vw2T_sb):
                nc.gpsimd.tensor_copy(t[0:1, 0, 0:1], h0[0:1, 0, 0:1])
            nc.gpsimd.tensor_copy(xbf_tiles[1][0:1, 2, 0:1], h0[0:1, 0, 0:1])
            nc.sync.dma_start(
                out=xbf_tiles[1][:, 0:2, :], in_=xbf_ext.ap()[1][:, 0:2, :]
            )
            nc.gpsimd.dma_start(
                out=xbf_tiles[1][:, 2:KT, :], in_=xbf_ext.ap()[1][:, 2:KT, :]
            )
            nc.gpsimd.dma_start(out=vw2T_sb, in_=vw2T_ext.ap())
            nc.sync.dma_start(out=x_tiles[0], in_=xt_ext.ap()[0])
            nc.gpsimd.dma_start(out=x_tiles[1], in_=xt_ext.ap()[1])
            gp0 = gp_gemm(0, h0)
            e0 = scores_exp(0, h0, gp0)
            s2f1 = gn_stats_pre(1)          # DVE during scoresT0 (free slot)
            mr1 = gn_stats(1, s2f1)         # DVE smalls ahead of recip0
            h1 = gn_apply(1, mr1, engines="gggg")   # GpSimd, keeps DVE/ACT free
            rS0 = softmax_S(0, e0)
            ut0 = ut_gemm(0, h0)            # fills PE while recip0 runs
            rsb0 = softmax_bc(0, rS0)
            attn_out_resid(0, ut0, e0, rsb0, x_tiles[0])
            gp1 = gp_gemm(1, h1)
            e1 = scores_exp(1, h1, gp1)
            rS1 = softmax_S(1, e1)
            ut1 = ut_gemm(1, h1)
            rsb1 = softmax_bc(1, rS1)
            attn_out_resid(1, ut1, e1, rsb1, x_tiles[1])

    nc.compile()
    return nc


def _get_nc():
    if "nc" not in _BUILD_CACHE:
        _BUILD_CACHE["nc"] = _build()
    return _BUILD_CACHE["nc"]


def _reference_fallback(x, norm_w, norm_b, qkv_w, qkv_b, proj_w, proj_b):
    """NumPy reference path (only for nonzero qkv/proj biases, which this
    problem's inputs never have)."""
    xf = x.reshape(B, C, N).astype(np.float64)
    xg = xf.reshape(B, G, GS, N)
    mu = xg.mean(axis=(2, 3), keepdims=True)
    var = xg.var(axis=(2, 3), keepdims=True)
    h = ((xg - mu) / np.sqrt(var + EPS)).reshape(B, C, N)
    h = h * norm_w[None, :, None] + norm_b[None, :, None]
    qkv = np.einsum("oc,bcn->bon", qkv_w.astype(np.float64), h) + qkv_b[None, :, None]
    q, k, v = qkv[:, :C], qkv[:, C : 2 * C], qkv[:, 2 * C :]
    z = np.einsum("bci,bcj->bij", q, k) * SCALE
    z -= z.max(axis=2, keepdims=True)
    e = np.exp(z)
    p = e / e.sum(axis=2, keepdims=True)
    o = np.einsum("bcj,bij->bci", v, p)
    out = xf + np.einsum("oc,bcn->bon", proj_w.astype(np.float64), o) + proj_b[None, :, None]
    return out.reshape(B, C, H, W).astype(np.float32)


def _tile_weight(w):
    """[C, C] -> [128, KT*C] with (p, kt*C + c) = w[kt*128+p, c]."""
    return np.ascontiguousarray(
        w.reshape(KT, 128, C).transpose(1, 0, 2).reshape(128, KT * C)
    )


def kernel(x, norm_w, norm_b, qkv_w, qkv_b, proj_w, proj_b, _trace=False):
    global LAST_RESULT

    x = np.asarray(x, dtype=np.float32)
    norm_w = np.asarray(norm_w, dtype=np.float32)
    norm_b = np.asarray(norm_b, dtype=np.float32)
    qkv_w = np.asarray(qkv_w, dtype=np.float32)
    qkv_b = np.asarray(qkv_b, dtype=np.float32)
    proj_w = np.asarray(proj_w, dtype=np.float32)
    proj_b = np.asarray(proj_b, dtype=np.float32)

    if np.any(qkv_b != 0.0) or np.any(proj_b != 0.0):
        return _reference_fallback(
            x, norm_w, norm_b, qkv_w, qkv_b, proj_w, proj_b
        )

    nc = _get_nc()

    # pre-tile x to [B, 128, KT, N] so each sample loads as one descriptor
    xt = np.ascontiguousarray(
        x.reshape(B, KT, 128, N).transpose(0, 2, 1, 3)
    )
    xbf = xt.astype(ml_dtypes.bfloat16)
    Wq = qkv_w[:C].astype(np.float64)
    Wk = qkv_w[C : 2 * C].astype(np.float64)
    Wv = qkv_w[2 * C :].astype(np.float64)
    P = proj_w.astype(np.float64)
    scw = _tile_weight((Wq.T @ Wk).astype(np.float32)).astype(ml_dtypes.bfloat16)
    vw2T = _tile_weight((P @ Wv).T.astype(np.float32)).astype(ml_dtypes.bfloat16)

    ind16 = np.zeros((128, 8), dtype=np.float32)
    for p in range(128):
        ind16[p, p // GS] = 1.0
    ind16T = np.ascontiguousarray(ind16.T)
    ind16 = ind16 * (1.0 / GS)  # fold the group-mean divide into the reduce
    cst17 = np.ascontiguousarray(
        np.concatenate(
            [
                norm_w.reshape(KT, 128).T,
                norm_b.reshape(KT, 128).T,
                ind16,
                np.ones((128, 1), dtype=np.float32),
            ],
            axis=1,
        )
    )
    shared = {
        "scw": scw,
        "vw2T": vw2T,
        "cst17": cst17,
        "ind16T": ind16T,
        "ones_f": np.ones(128, dtype=np.float32),
    }
    in_maps = [
        {
            "xt": np.ascontiguousarray(xt[c * SPC : (c + 1) * SPC]),
            "xbf": np.ascontiguousarray(xbf[c * SPC : (c + 1) * SPC]),
            **shared,
        }
        for c in range(NCORES)
    ]
    res = run_bass_kernel_spmd(nc, in_maps, list(range(NCORES)), trace=_trace)
    LAST_RESULT = res
    out = np.concatenate([res.results[i]["y"] for i in range(NCORES)], axis=0)
    return out.reshape(B, C, H, W)
